# revision 8
# baseline (speedup 1.0000x reference)
"""Trainium2 Bass kernel for nn_AttentionSimple (sparse_attention, 8 cores).

Reference (per batch row b):
    e      = embeddings[k[b]]              # [S, E] gather
    scores = q[b] . e[s]                   # [S]
    attn   = softmax(scores); ctx = sum_s attn[s] * e[s]
    out    = ctx @ W.T + b                 # [B, 2]

Algorithm: count-weighted vocab-space softmax - no per-token gathers.
Scores depend on s only through v = k[b, s], so group softmax terms by
vocabulary id:
    c[b, v]  = |{s : k[b, s] = v}|         (histogram of k, built host-side
                                            during input sharding)
    l[b, v]  = q[b] . embeddings[v]        (dense PE matmul, fp16 inputs)
    A        = exp(l)                      (ACT, fp32 -> bf16)
    out[b]   = (sum_v c*A*EW[v]) / (sum_v c*A[b,v]),  EW = emb @ W.T + b

Sharding: padded vocabulary (51200 = 8 x 50 chunks of 128) split across 8
cores; each core handles all 128 batch rows for its 6400 vocab entries.
Each core returns partial numerators/denominators; host sums and divides.

Per-core pipeline (v2):
  - mm1: 25 pair matmuls. lhsT = fp16 pair block (embedding rows 0:50 =
    even chunk, 50:100 = odd chunk), rhs = block-diagonal [qT|qT] fp16,
    N=256 -> one matmul yields two chunks' logits for all 128 batches.
  - ACT: exp over 3-PSUM-bank spans (1536 cols) -> bf16 A tiles; big
    tiles amortize the ~293ns per-instruction overhead.
  - DVE: A *= counts (fp8_e4m3 transport - exact for counts <= 15).
  - mm2: acc[9, 512] += st9_quad.T @ A (bf16), 13 accumulating matmuls.
  - DMA: 7 dispatches total (fatter transfers; the v1 13-dispatch stream
    ran at 148 GB/s, dispatch-serialization-limited).
  - Emission order interleaves mm1 groups ahead of mm2 so the PE queue
    (strict in-order) never stalls on the ACT->DVE chain.
"""

import numpy as np

BATCH, SEQ, EMB, VOCAB, OUT = 128, 8192, 50, 50000, 2
N_CORES = 8
CSH = 50                         # vocab chunks per core
NCHUNK = CSH * N_CORES           # 400
VPAD = NCHUNK * 128              # 51200
VSH = CSH * 128                  # 6400
NPAIR = CSH // 2                 # 25
NQUAD = 13                       # 12 full quads + 1 pair-quad
GROUPS = [6, 6, 6, 6, 1]         # pairs per processing group

_CACHE = {}


def _build_nc():
    from contextlib import ExitStack

    import concourse.mybir as mybir
    import concourse.tile as tile
    from concourse import bacc

    f32 = mybir.dt.float32
    f16 = mybir.dt.float16
    bf16 = mybir.dt.bfloat16
    fp8 = mybir.dt.float8e4
    nc = bacc.Bacc("TRN2", target_bir_lowering=False, debug=False,
                   num_devices=N_CORES)

    et_d = nc.dram_tensor("et", [128, NPAIR * 128], f16, kind="ExternalInput")
    qw_d = nc.dram_tensor("qw", [128, 256], f16, kind="ExternalInput")
    st_d = nc.dram_tensor("st", [128, NQUAD * 9], bf16, kind="ExternalInput")
    ct_d = nc.dram_tensor("ct", [128, CSH * BATCH], bf16, kind="ExternalInput")
    o_d = nc.dram_tensor("o", [9, 4 * BATCH], f32, kind="ExternalOutput")

    with tile.TileContext(nc) as tc, ExitStack() as ctx:
        const_p = ctx.enter_context(tc.tile_pool(name="const", bufs=1))
        ps_p = ctx.enter_context(tc.tile_pool(name="ps", bufs=2, space="PSUM"))
        tail_p = ctx.enter_context(tc.tile_pool(name="tailps", bufs=1,
                                                space="PSUM"))
        acc_p = ctx.enter_context(tc.tile_pool(name="acc", bufs=1,
                                               space="PSUM"))

        # SBUF tiles (one tile per DMA so dependency tracking stays exact)
        wtile = const_p.tile([128, 256], f16)
        qw_sb = const_p.tile([128, 256], f16)
        st_sb = const_p.tile([128, NQUAD * 9], bf16)
        etA = const_p.tile([128, 768], f16)     # pairs 0-5
        etB = const_p.tile([128, 768], f16)     # pairs 6-11
        etC = const_p.tile([128, 1664], f16)    # pairs 12-24
        ctA = const_p.tile([128, 3072], bf16)   # groups 0-1
        ctB = const_p.tile([128, 3328], bf16)   # groups 2-4
        a_tiles = [const_p.tile([128, g * 256], bf16, name=f"a{i}")
                   for i, g in enumerate(GROUPS)]

        nc.gpsimd.memset(wtile[:].bitcast(f32), 0.0)

        # Scalar queue: params, then counts (needed from the DVE stage on)
        nc.scalar.dma_start(qw_sb[:], qw_d.ap())
        nc.scalar.dma_start(st_sb[:], st_d.ap())
        nc.scalar.dma_start(ctA[:], ct_d.ap()[:, 0:3072])
        nc.scalar.dma_start(ctB[:], ct_d.ap()[:, 3072:6400])

        # Sync queue: embeddings in consumption order
        nc.sync.dma_start(etA[:], et_d.ap()[:, 0:768])
        nc.sync.dma_start(etB[:], et_d.ap()[:, 768:1536])
        nc.sync.dma_start(etC[:], et_d.ap()[:, 1536:3200])

        # PE warm-up while the first DMAs land
        wps = tail_p.tile([128, 256], f32, tag="tail")
        for _ in range(6):
            nc.tensor.matmul(wps[:], lhsT=wtile[:, 0:128], rhs=wtile[:],
                             start=True, stop=True)

        acc = acc_p.tile([9, 4 * BATCH], f32)

        et_of_group = [etA, etB, etC, etC, etC]
        et_col0 = [0, 0, 0, 768, 1536]          # col offset within its tile
        ct_of_group = [ctA, ctA, ctB, ctB, ctB]
        ct_col0 = [0, 1536, 0, 1536, 3072]

        ps_tiles = [None] * len(GROUPS)

        def emit_mm1(g):
            npairs = GROUPS[g]
            if g == len(GROUPS) - 1:
                ps = tail_p.tile([128, 256], f32, tag="tail")
            else:
                ps = ps_p.tile([128, 1536], f32, tag="ps")
            ps_tiles[g] = ps
            et = et_of_group[g]
            c0 = et_col0[g]
            for p in range(npairs):
                nc.tensor.matmul(
                    ps[:, p * 256:(p + 1) * 256],
                    lhsT=et[:, c0 + p * 128:c0 + (p + 1) * 128],
                    rhs=qw_sb[:],
                    start=True, stop=True,
                )

        quad_idx = 0

        def emit_tail(g):
            nonlocal quad_idx
            npairs = GROUPS[g]
            ncols = npairs * 256
            ps = ps_tiles[g]
            a = a_tiles[g]
            if g == 0:
                # split: exp of the first 2 pairs starts the ACT chain
                # as soon as those matmuls land
                nc.scalar.activation(a[:, 0:512], ps[:, 0:512],
                                     mybir.ActivationFunctionType.Exp)
                nc.scalar.activation(a[:, 512:ncols], ps[:, 512:ncols],
                                     mybir.ActivationFunctionType.Exp)
            else:
                nc.scalar.activation(a[:], ps[:, 0:ncols],
                                     mybir.ActivationFunctionType.Exp)
            ct = ct_of_group[g]
            c0 = ct_col0[g]
            nquads = (npairs + 1) // 2
            for j in range(nquads):
                n = min(512, ncols - j * 512)
                # per-quad count-multiply (bf16 x bf16 -> 2x DVE mode),
                # so mm2 of quad j starts without waiting the whole group
                nc.vector.tensor_mul(a[:, j * 512:j * 512 + n],
                                     a[:, j * 512:j * 512 + n],
                                     ct[:, c0 + j * 512:c0 + j * 512 + n])
                nc.tensor.matmul(
                    acc[:, 0:n],
                    lhsT=st_sb[:, quad_idx * 9:(quad_idx + 1) * 9],
                    rhs=a[:, j * 512:j * 512 + n],
                    start=(quad_idx == 0), stop=(quad_idx == NQUAD - 1),
                    skip_group_check=True,
                )
                quad_idx += 1

        # Interleave: PE program order keeps mm1 ahead of the mm2 that
        # depends on the ACT->DVE chain.
        emit_mm1(0)
        emit_mm1(1)
        emit_tail(0)
        emit_mm1(2)
        emit_tail(1)
        emit_mm1(3)
        emit_tail(2)
        emit_mm1(4)
        emit_tail(3)
        emit_tail(4)

        osb = const_p.tile([9, 4 * BATCH], f32)
        nc.vector.tensor_copy(osb[:], acc[:])
        nc.scalar.dma_start(o_d.ap(), osb[:])

    nc.finalize()
    return nc


def _prep_inputs(q, k, embeddings, W, b):
    import ml_dtypes

    q = np.ascontiguousarray(q, dtype=np.float32)
    emb = np.ascontiguousarray(embeddings, dtype=np.float32)
    W = np.ascontiguousarray(W, dtype=np.float32)
    b = np.ascontiguousarray(b, dtype=np.float32)
    k = np.asarray(k)

    embT = np.zeros((EMB, VPAD), np.float32)
    embT[:, :VOCAB] = emb.T

    qw = np.zeros((128, 256), np.float16)
    qw[0:EMB, 0:BATCH] = q.T
    qw[EMB:2 * EMB, BATCH:2 * BATCH] = q.T

    EWp = np.zeros((VPAD, OUT), np.float32)
    EWp[:VOCAB] = emb @ W.T + b[None, :]

    flat = (np.arange(BATCH, dtype=np.int64)[:, None] * VPAD
            + k.astype(np.int64)).ravel()
    C = np.bincount(flat, minlength=BATCH * VPAD).reshape(BATCH, VPAD)
    assert C.max() <= 256, "count histogram overflows bf16 exact range"
    C = C.astype(np.float32)

    in_maps = []
    for core in range(N_CORES):
        v0 = core * VSH
        blocks = embT[:, v0:v0 + VSH].reshape(EMB, CSH, 128)
        e2 = np.zeros((128, NPAIR * 128), np.float16)
        e2[0:EMB] = blocks[:, 0::2, :].reshape(EMB, NPAIR * 128)
        e2[EMB:2 * EMB] = blocks[:, 1::2, :].reshape(EMB, NPAIR * 128)

        ew = EWp[v0:v0 + VSH].reshape(CSH, 128, OUT)
        st = np.zeros((128, NQUAD, 9), np.float32)
        for qd in range(12):
            for j in range(4):
                st[:, qd, 2 * j:2 * j + 2] = ew[4 * qd + j]
            st[:, qd, 8] = 1.0
        st[:, 12, 0:2] = ew[48]
        st[:, 12, 2:4] = ew[49]
        st[:, 12, 8] = 1.0
        st = np.ascontiguousarray(
            st.reshape(128, NQUAD * 9)).astype(ml_dtypes.bfloat16)

        ct = np.ascontiguousarray(
            C[:, v0:v0 + VSH].reshape(BATCH, CSH, 128)
            .transpose(2, 1, 0).reshape(128, CSH * BATCH)
            .astype(ml_dtypes.bfloat16))
        in_maps.append({"et": np.ascontiguousarray(e2), "qw": qw,
                        "st": st, "ct": ct})
    return in_maps


def _run_device(in_maps, **kwargs):
    from concourse.bass_utils import run_bass_kernel_spmd

    if "nc" not in _CACHE:
        _CACHE["nc"] = _build_nc()
    return run_bass_kernel_spmd(_CACHE["nc"], in_maps,
                                core_ids=list(range(N_CORES)), **kwargs)


def _unshard(res):
    P = np.zeros((9, 4 * BATCH), np.float64)
    for i in range(N_CORES):
        P += res.results[i]["o"].astype(np.float64)
    numer = np.zeros((OUT, BATCH), np.float64)
    denom = np.zeros(BATCH, np.float64)
    for j in range(4):
        numer += P[2 * j:2 * j + 2, j * BATCH:(j + 1) * BATCH]
        denom += P[8, j * BATCH:(j + 1) * BATCH]
    out = (numer / denom[None, :]).T
    return np.ascontiguousarray(out, dtype=np.float32)


def kernel(q, k, embeddings, W, b, **_unused):
    in_maps = _prep_inputs(q, k, embeddings, W, b)
    res = _run_device(in_maps)
    return _unshard(res)


# revision 9
# speedup vs baseline: 1.0190x; 1.0190x over previous
"""Trainium2 Bass kernel for nn_AttentionSimple (sparse_attention, 8 cores).

Reference (per batch row b):
    e      = embeddings[k[b]]              # [S, E] gather
    scores = q[b] . e[s]                   # [S]
    attn   = softmax(scores); ctx = sum_s attn[s] * e[s]
    out    = ctx @ W.T + b                 # [B, 2]

Algorithm: count-weighted vocab-space softmax - no per-token gathers.
Scores depend on s only through v = k[b, s], so group softmax terms by
vocabulary id:
    c[b, v]  = |{s : k[b, s] = v}|         (histogram of k, built host-side
                                            during input sharding)
    l[b, v]  = q[b] . embeddings[v]        (dense PE matmul, fp16 inputs)
    A        = exp(l)                      (ACT, fp32 -> bf16)
    out[b]   = (sum_v c*A*EW[v]) / (sum_v c*A[b,v]),  EW = emb @ W.T + b

Sharding: padded vocabulary (51200 = 8 x 50 chunks of 128) split across 8
cores; each core handles all 128 batch rows for its 6400 vocab entries.
Each core returns partial numerators/denominators; host sums and divides.

Per-core pipeline (v2):
  - mm1: 25 pair matmuls. lhsT = fp16 pair block (embedding rows 0:50 =
    even chunk, 50:100 = odd chunk), rhs = block-diagonal [qT|qT] fp16,
    N=256 -> one matmul yields two chunks' logits for all 128 batches.
  - ACT: exp over 3-PSUM-bank spans (1536 cols) -> bf16 A tiles; big
    tiles amortize the ~293ns per-instruction overhead.
  - DVE: A *= counts (fp8_e4m3 transport - exact for counts <= 15).
  - mm2: acc[9, 512] += st9_quad.T @ A (bf16), 13 accumulating matmuls.
  - DMA: 7 dispatches total (fatter transfers; the v1 13-dispatch stream
    ran at 148 GB/s, dispatch-serialization-limited).
  - Emission order interleaves mm1 groups ahead of mm2 so the PE queue
    (strict in-order) never stalls on the ACT->DVE chain.
"""

import numpy as np

BATCH, SEQ, EMB, VOCAB, OUT = 128, 8192, 50, 50000, 2
N_CORES = 8
CSH = 50                         # vocab chunks per core
NCHUNK = CSH * N_CORES           # 400
VPAD = NCHUNK * 128              # 51200
VSH = CSH * 128                  # 6400
NPAIR = CSH // 2                 # 25
NQUAD = 13                       # 12 full quads + 1 pair-quad
GROUPS = [6, 6, 6, 6, 1]         # pairs per processing group

_CACHE = {}


def _build_nc():
    from contextlib import ExitStack

    import concourse.mybir as mybir
    import concourse.tile as tile
    from concourse import bacc

    f32 = mybir.dt.float32
    f16 = mybir.dt.float16
    bf16 = mybir.dt.bfloat16
    fp8 = mybir.dt.float8e4
    nc = bacc.Bacc("TRN2", target_bir_lowering=False, debug=False,
                   num_devices=N_CORES)

    et_d = nc.dram_tensor("et", [128, NPAIR * 128], f16, kind="ExternalInput")
    qw_d = nc.dram_tensor("qw", [128, 256], f16, kind="ExternalInput")
    st_d = nc.dram_tensor("st", [128, NQUAD * 9], bf16, kind="ExternalInput")
    ct_d = nc.dram_tensor("ct", [128, CSH * BATCH], bf16, kind="ExternalInput")
    o_d = nc.dram_tensor("o", [9, 4 * BATCH], f32, kind="ExternalOutput")

    with tile.TileContext(nc) as tc, ExitStack() as ctx:
        const_p = ctx.enter_context(tc.tile_pool(name="const", bufs=1))
        ps_p = ctx.enter_context(tc.tile_pool(name="ps", bufs=2, space="PSUM"))
        tail_p = ctx.enter_context(tc.tile_pool(name="tailps", bufs=1,
                                                space="PSUM"))
        acc_p = ctx.enter_context(tc.tile_pool(name="acc", bufs=1,
                                               space="PSUM"))

        # SBUF tiles (one tile per DMA so dependency tracking stays exact)
        wtile = const_p.tile([128, 256], f16)
        qw_sb = const_p.tile([128, 256], f16)
        st_sb = const_p.tile([128, NQUAD * 9], bf16)
        etA = const_p.tile([128, 768], f16)     # pairs 0-5
        etB = const_p.tile([128, 768], f16)     # pairs 6-11
        etC = const_p.tile([128, 1664], f16)    # pairs 12-24
        ctA = const_p.tile([128, 3072], bf16)   # groups 0-1
        ctB = const_p.tile([128, 3328], bf16)   # groups 2-4
        a_tiles = [const_p.tile([128, g * 256], bf16, name=f"a{i}")
                   for i, g in enumerate(GROUPS)]

        nc.gpsimd.memset(wtile[:].bitcast(f32), 0.0)

        # Scalar queue: params, then counts (needed from the DVE stage on)
        nc.scalar.dma_start(qw_sb[:], qw_d.ap())
        nc.scalar.dma_start(st_sb[:], st_d.ap())
        nc.scalar.dma_start(ctA[:], ct_d.ap()[:, 0:3072])
        nc.scalar.dma_start(ctB[:], ct_d.ap()[:, 3072:6400])

        # Sync queue: embeddings in consumption order
        nc.sync.dma_start(etA[:], et_d.ap()[:, 0:768])
        nc.sync.dma_start(etB[:], et_d.ap()[:, 768:1536])
        nc.sync.dma_start(etC[:], et_d.ap()[:, 1536:3200])

        # PE warm-up while the first DMAs land
        wps = tail_p.tile([128, 256], f32, tag="tail")
        for _ in range(6):
            nc.tensor.matmul(wps[:], lhsT=wtile[:, 0:128], rhs=wtile[:],
                             start=True, stop=True)

        acc = acc_p.tile([9, 4 * BATCH], f32)

        et_of_group = [etA, etB, etC, etC, etC]
        et_col0 = [0, 0, 0, 768, 1536]          # col offset within its tile
        ct_of_group = [ctA, ctA, ctB, ctB, ctB]
        ct_col0 = [0, 1536, 0, 1536, 3072]

        ps_tiles = [None] * len(GROUPS)

        def emit_mm1(g):
            npairs = GROUPS[g]
            if g == len(GROUPS) - 1:
                ps = tail_p.tile([128, 256], f32, tag="tail")
            else:
                ps = ps_p.tile([128, 1536], f32, tag="ps")
            ps_tiles[g] = ps
            et = et_of_group[g]
            c0 = et_col0[g]
            for p in range(npairs):
                nc.tensor.matmul(
                    ps[:, p * 256:(p + 1) * 256],
                    lhsT=et[:, c0 + p * 128:c0 + (p + 1) * 128],
                    rhs=qw_sb[:],
                    start=True, stop=True,
                )

        quad_idx = 0

        def emit_tail(g):
            nonlocal quad_idx
            npairs = GROUPS[g]
            ncols = npairs * 256
            ps = ps_tiles[g]
            a = a_tiles[g]
            nc.scalar.activation(a[:], ps[:, 0:ncols],
                                 mybir.ActivationFunctionType.Exp)
            ct = ct_of_group[g]
            c0 = ct_col0[g]
            nquads = (npairs + 1) // 2
            for j in range(nquads):
                n = min(512, ncols - j * 512)
                # per-quad count-multiply (bf16 x bf16 -> 2x DVE mode),
                # so mm2 of quad j starts without waiting the whole group
                nc.vector.tensor_mul(a[:, j * 512:j * 512 + n],
                                     a[:, j * 512:j * 512 + n],
                                     ct[:, c0 + j * 512:c0 + j * 512 + n])
                nc.tensor.matmul(
                    acc[:, 0:n],
                    lhsT=st_sb[:, quad_idx * 9:(quad_idx + 1) * 9],
                    rhs=a[:, j * 512:j * 512 + n],
                    start=(quad_idx == 0), stop=(quad_idx == NQUAD - 1),
                    skip_group_check=True,
                )
                quad_idx += 1

        # Interleave: PE program order keeps mm1 ahead of the mm2 that
        # depends on the ACT->DVE chain.
        emit_mm1(0)
        emit_mm1(1)
        emit_tail(0)
        emit_mm1(2)
        emit_tail(1)
        emit_mm1(3)
        emit_tail(2)
        emit_mm1(4)
        emit_tail(3)
        emit_tail(4)

        osb = const_p.tile([9, 4 * BATCH], f32)
        nc.vector.tensor_copy(osb[:], acc[:])
        nc.scalar.dma_start(o_d.ap(), osb[:])

    nc.finalize()
    return nc


def _prep_inputs(q, k, embeddings, W, b):
    import ml_dtypes

    q = np.ascontiguousarray(q, dtype=np.float32)
    emb = np.ascontiguousarray(embeddings, dtype=np.float32)
    W = np.ascontiguousarray(W, dtype=np.float32)
    b = np.ascontiguousarray(b, dtype=np.float32)
    k = np.asarray(k)

    embT = np.zeros((EMB, VPAD), np.float32)
    embT[:, :VOCAB] = emb.T

    qw = np.zeros((128, 256), np.float16)
    qw[0:EMB, 0:BATCH] = q.T
    qw[EMB:2 * EMB, BATCH:2 * BATCH] = q.T

    EWp = np.zeros((VPAD, OUT), np.float32)
    EWp[:VOCAB] = emb @ W.T + b[None, :]

    flat = (np.arange(BATCH, dtype=np.int64)[:, None] * VPAD
            + k.astype(np.int64)).ravel()
    C = np.bincount(flat, minlength=BATCH * VPAD).reshape(BATCH, VPAD)
    assert C.max() <= 256, "count histogram overflows bf16 exact range"
    C = C.astype(np.float32)

    in_maps = []
    for core in range(N_CORES):
        v0 = core * VSH
        blocks = embT[:, v0:v0 + VSH].reshape(EMB, CSH, 128)
        e2 = np.zeros((128, NPAIR * 128), np.float16)
        e2[0:EMB] = blocks[:, 0::2, :].reshape(EMB, NPAIR * 128)
        e2[EMB:2 * EMB] = blocks[:, 1::2, :].reshape(EMB, NPAIR * 128)

        ew = EWp[v0:v0 + VSH].reshape(CSH, 128, OUT)
        st = np.zeros((128, NQUAD, 9), np.float32)
        for qd in range(12):
            for j in range(4):
                st[:, qd, 2 * j:2 * j + 2] = ew[4 * qd + j]
            st[:, qd, 8] = 1.0
        st[:, 12, 0:2] = ew[48]
        st[:, 12, 2:4] = ew[49]
        st[:, 12, 8] = 1.0
        st = np.ascontiguousarray(
            st.reshape(128, NQUAD * 9)).astype(ml_dtypes.bfloat16)

        ct = np.ascontiguousarray(
            C[:, v0:v0 + VSH].reshape(BATCH, CSH, 128)
            .transpose(2, 1, 0).reshape(128, CSH * BATCH)
            .astype(ml_dtypes.bfloat16))
        in_maps.append({"et": np.ascontiguousarray(e2), "qw": qw,
                        "st": st, "ct": ct})
    return in_maps


def _run_device(in_maps, **kwargs):
    from concourse.bass_utils import run_bass_kernel_spmd

    if "nc" not in _CACHE:
        _CACHE["nc"] = _build_nc()
    return run_bass_kernel_spmd(_CACHE["nc"], in_maps,
                                core_ids=list(range(N_CORES)), **kwargs)


def _unshard(res):
    P = np.zeros((9, 4 * BATCH), np.float64)
    for i in range(N_CORES):
        P += res.results[i]["o"].astype(np.float64)
    numer = np.zeros((OUT, BATCH), np.float64)
    denom = np.zeros(BATCH, np.float64)
    for j in range(4):
        numer += P[2 * j:2 * j + 2, j * BATCH:(j + 1) * BATCH]
        denom += P[8, j * BATCH:(j + 1) * BATCH]
    out = (numer / denom[None, :]).T
    return np.ascontiguousarray(out, dtype=np.float32)


def kernel(q, k, embeddings, W, b, **_unused):
    in_maps = _prep_inputs(q, k, embeddings, W, b)
    res = _run_device(in_maps)
    return _unshard(res)


# revision 11
# speedup vs baseline: 1.2427x; 1.2196x over previous
"""Trainium2 Bass kernel for nn_AttentionSimple (sparse_attention, 8 cores).

Reference (per batch row b):
    e      = embeddings[k[b]]              # [S, E] gather
    scores = q[b] . e[s]                   # [S]
    attn   = softmax(scores); ctx = sum_s attn[s] * e[s]
    out    = ctx @ W.T + b                 # [B, 2]

Algorithm: count-weighted vocab-space softmax - no per-token gathers.
Scores depend on s only through v = k[b, s], so group softmax terms by
vocabulary id:
    c[b, v]  = |{s : k[b, s] = v}|         (histogram of k, host-side)
    l[b, v]  = q[b] . embeddings[v]        (dense PE matmul, fp16 inputs)
    out[b]   = (sum_v c*e^l*EW[v]) / (sum_v c*e^l),  EW = emb @ W.T + b

Sharding: padded vocabulary (51200 = 8 x 50 chunks of 128) split across 8
cores; each core handles all 128 batch rows for its 6400 vocab entries,
returning partial numerators/denominators; host sums and divides.

Per-core pipeline (v5):
  - mm1: 25 pair matmuls, fp16. lhsT = pair block (embedding rows 0:50 =
    even chunk, 50:100 = odd chunk, row 127 = 1), rhs = block-diagonal
    [qT|qT]; N=256 -> one matmul = two chunks' logits for 128 batches.
  - exp runs SPLIT across two engines (the ACT exp chain is otherwise the
    critical resource at ~(N+352)/1.2 ns per call):
      * ACT half (pairs 0-19): ACT exp PSUM->bf16 A; DVE multiplies by
        bf16 counts (2x 16-bit mode); mm2 in bf16.
      * DVE half (pairs 20-24): Schraudolph bit-trick exp fused with the
        count weighting in ONE 1x DVE op:
            i32 = (l + s)*K + D_c ;  bitcast i32 -> float ~= c * e^l
        building bf16 BITS via int16: K = 2^7/ln2, s = (127*2^7 - C)/K
        added during mm1 via an extra all-ones contraction row (qw2),
        D_c = 2^7*log2(c) (bf16, -inf for c=0 -> saturating convert ->
        INT16_MIN = bf16 -0.0). The i16 tile bitcasts to bf16 for mm2.
        The ~3% sawtooth error cancels between numerator & denominator
        for dominant softmax terms; simulated end-to-end err 5.4e-3.
  - mm2: acc[9, 512] += st_quad.T @ A, 13 accumulating matmuls (bf16 for
    ACT-half quads, f32r for the bitcast DVE-half quads). A separate
    zero matmul pre-clears acc so all mm2s are order-independent.
  - Params ride the gpsimd SWDGE queue, bulk data the sync HWDGE queue;
    the scalar queue carries ONLY the ACT chain (a dummy activation
    forces the exp table load at t~0).
"""

import numpy as np

BATCH, SEQ, EMB, VOCAB, OUT = 128, 8192, 50, 50000, 2
N_CORES = 8
CSH = 50                         # vocab chunks per core
VPAD = CSH * N_CORES * 128       # 51200
VSH = CSH * 128                  # 6400
NPAIR = 25
NQUAD = 13
NPAIR_ACT = 20                   # pairs 0-19 -> ACT exp half
NPAIR_DVE = 5                    # pairs 20-24 -> DVE Schraudolph half
K_SCHRAU = 2.0 ** 7 / np.log(2.0)           # 184.664 (bf16-bits variant)
C_SCHRAU = 5.5913
S_EXACT = (127.0 * 2.0 ** 7 - C_SCHRAU) / K_SCHRAU
S_F16 = float(np.float32(np.float16(S_EXACT)))
S_DELTA = S_EXACT - S_F16

_CACHE = {}


def _build_nc():
    from contextlib import ExitStack

    import concourse.mybir as mybir
    import concourse.tile as tile
    from concourse import bacc

    f32 = mybir.dt.float32
    f32r = mybir.dt.float32r
    f16 = mybir.dt.float16
    bf16 = mybir.dt.bfloat16
    i16 = mybir.dt.int16
    AF = mybir.ActivationFunctionType
    nc = bacc.Bacc("TRN2", target_bir_lowering=False, debug=False,
                   num_devices=N_CORES)

    et_d = nc.dram_tensor("et", [128, NPAIR * 128], f16, kind="ExternalInput")
    qw_d = nc.dram_tensor("qw", [128, 256], f16, kind="ExternalInput")
    qw2_d = nc.dram_tensor("qw2", [128, 256], f16, kind="ExternalInput")
    stA_d = nc.dram_tensor("stA", [128, NQUAD * 9], bf16,
                           kind="ExternalInput")
    ct_d = nc.dram_tensor("ct", [128, NPAIR_ACT * 256], bf16,
                          kind="ExternalInput")
    dd_d = nc.dram_tensor("dd", [128, NPAIR_DVE * 256], bf16,
                          kind="ExternalInput")
    o_d = nc.dram_tensor("o", [9, 4 * BATCH], f32, kind="ExternalOutput")

    with tile.TileContext(nc) as tc, ExitStack() as ctx:
        const_p = ctx.enter_context(tc.tile_pool(name="const", bufs=1))
        ps_p = ctx.enter_context(tc.tile_pool(name="ps", bufs=2, space="PSUM"))
        tail_p = ctx.enter_context(tc.tile_pool(name="tailps", bufs=1,
                                                space="PSUM"))
        acc_p = ctx.enter_context(tc.tile_pool(name="acc", bufs=1,
                                               space="PSUM"))

        wtile = const_p.tile([128, 512], f16)
        dumo = const_p.tile([128, 1], bf16)
        qw_sb = const_p.tile([128, 256], f16)
        qw2_sb = const_p.tile([128, 256], f16)
        stA_sb = const_p.tile([128, NQUAD * 9], bf16)
        et0 = const_p.tile([128, 256], f16)     # pairs 0-1   (group 0)
        et1 = const_p.tile([128, 768], f16)     # pairs 2-7   (group 1)
        etC = const_p.tile([128, 2176], f16)    # pairs 8-24  (groups 2-5)
        ct0 = const_p.tile([128, 512], bf16)    # counts, group 0
        ct1 = const_p.tile([128, 1536], bf16)   # counts, group 1
        ct23 = const_p.tile([128, 3072], bf16)  # counts, groups 2-3
        dd = const_p.tile([128, 1280], bf16)    # Schraudolph D, groups 4-5
        a0 = const_p.tile([128, 512], bf16)
        a1 = const_p.tile([128, 1536], bf16)
        a2 = const_p.tile([128, 1536], bf16)
        a3 = const_p.tile([128, 1536], bf16)
        a4 = const_p.tile([128, 1024], i16)
        a5 = const_p.tile([128, 256], i16)
        osb = const_p.tile([9, 4 * BATCH], f32)

        nc.gpsimd.memset(wtile[:].bitcast(f32), 0.0)

        # Scalar queue: ONLY the ACT chain. The dummy activation forces
        # walrus to place the exp table load at kernel start.
        nc.scalar.activation(dumo[:], wtile[:, 0:1], AF.Exp)

        # gpsimd SWDGE queue: small parameter tensors
        nc.gpsimd.dma_start(qw_sb[:], qw_d.ap())
        nc.gpsimd.dma_start(qw2_sb[:], qw2_d.ap())
        nc.gpsimd.dma_start(stA_sb[:], stA_d.ap())

        # Sync HWDGE queue: bulk transfers in consumption order
        nc.sync.dma_start(et0[:], et_d.ap()[:, 0:256])
        nc.sync.dma_start(et1[:], et_d.ap()[:, 256:1024])
        nc.sync.dma_start(ct0[:], ct_d.ap()[:, 0:512])
        nc.sync.dma_start(etC[:], et_d.ap()[:, 1024:3200])
        nc.sync.dma_start(ct1[:], ct_d.ap()[:, 512:2048])
        nc.sync.dma_start(ct23[:], ct_d.ap()[:, 2048:5120])
        nc.sync.dma_start(dd[:], dd_d.ap())

        acc = acc_p.tile([9, 4 * BATCH], f32)
        # pre-clear acc so every real mm2 can use start=False (the PE
        # stream stays reorder-safe: accumulation commutes)
        nc.tensor.matmul(acc[:], lhsT=wtile[:, 0:9], rhs=wtile[:],
                         start=True, stop=False, skip_group_check=True)
        # HAM warm-up: keep the PE busy until the first data lands
        wps = tail_p.tile([128, 512], f32, tag="tail")
        for _ in range(9):
            nc.tensor.matmul(wps[:, 0:256], lhsT=wtile[:, 0:128],
                             rhs=wtile[:, 0:256], start=True, stop=True)

        # groups: (npairs, kind, et tile, et col0, qw, ct/D tile, col0)
        GROUPS = [
            (2, "act", et0, 0, qw_sb, ct0, 0),
            (6, "act", et1, 0, qw_sb, ct1, 0),
            (6, "act", etC, 0, qw_sb, ct23, 0),
            (6, "act", etC, 768, qw_sb, ct23, 1536),
            (4, "dve", etC, 1536, qw2_sb, dd, 0),
            (1, "dve", etC, 2048, qw2_sb, dd, 1024),
        ]
        A_TILES = [a0, a1, a2, a3, a4, a5]
        ps_tiles = [None] * 6
        quad0 = [0, 1, 4, 7, 10, 12]   # first global quad of each group

        def emit_mm1(g):
            npairs, kind, et, c0, qw, _, _ = GROUPS[g]
            if g in (0, 5):
                ps = tail_p.tile([128, 512], f32, tag="tail")
            else:
                ps = ps_p.tile([128, 1536], f32, tag="ps")
            ps_tiles[g] = ps
            for p in range(npairs):
                nc.tensor.matmul(
                    ps[:, p * 256:(p + 1) * 256],
                    lhsT=et[:, c0 + p * 128:c0 + (p + 1) * 128],
                    rhs=qw[:],
                    start=True, stop=True,
                )

        def emit_exp(g):
            npairs, kind, _, _, _, cd, cd0 = GROUPS[g]
            ncols = npairs * 256
            ps = ps_tiles[g]
            a = A_TILES[g]
            if kind == "act":
                nc.scalar.activation(a[:], ps[:, 0:ncols], AF.Exp)
            else:
                nc.vector.scalar_tensor_tensor(
                    a[:], ps[:, 0:ncols], float(K_SCHRAU),
                    cd[:, cd0:cd0 + ncols],
                    op0=mybir.AluOpType.mult, op1=mybir.AluOpType.add)

        def emit_mm2(g):
            npairs, kind, _, _, _, cd, cd0 = GROUPS[g]
            ncols = npairs * 256
            a = A_TILES[g]
            nquads = (npairs + 1) // 2
            for j in range(nquads):
                n = min(512, ncols - j * 512)
                qd = quad0[g] + j
                if kind == "act":
                    nc.vector.tensor_mul(
                        a[:, j * 512:j * 512 + n],
                        a[:, j * 512:j * 512 + n],
                        cd[:, cd0 + j * 512:cd0 + j * 512 + n])
                    rhs = a[:, j * 512:j * 512 + n]
                else:
                    rhs = a[:, j * 512:j * 512 + n].bitcast(bf16)
                nc.tensor.matmul(acc[:, 0:n],
                                 lhsT=stA_sb[:, qd * 9:(qd + 1) * 9],
                                 rhs=rhs,
                                 start=False, stop=(qd == NQUAD - 1),
                                 skip_group_check=True)

        emit_mm1(0)
        emit_mm1(1)
        emit_exp(0)
        emit_mm2(0)
        emit_mm1(2)
        emit_exp(1)
        emit_mm2(1)
        emit_mm1(3)
        emit_exp(2)
        emit_mm2(2)
        emit_mm1(4)
        emit_mm1(5)
        emit_exp(4)        # DVE STT: ahead of group-3 muls in the queue
        emit_exp(3)
        emit_mm2(4)
        emit_exp(5)
        emit_mm2(3)
        emit_mm2(5)

        nc.scalar.activation(osb[:], acc[:], AF.Copy)
        nc.sync.dma_start(o_d.ap(), osb[:])

    nc.finalize()
    return nc


def _prep_inputs(q, k, embeddings, W, b):
    import ml_dtypes

    q = np.ascontiguousarray(q, dtype=np.float32)
    emb = np.ascontiguousarray(embeddings, dtype=np.float32)
    W = np.ascontiguousarray(W, dtype=np.float32)
    b = np.ascontiguousarray(b, dtype=np.float32)
    k = np.asarray(k)

    embT = np.zeros((EMB, VPAD), np.float32)
    embT[:, :VOCAB] = emb.T

    qw = np.zeros((128, 256), np.float16)
    qw[0:EMB, 0:BATCH] = q.T
    qw[EMB:2 * EMB, BATCH:2 * BATCH] = q.T
    qw2 = qw.copy()
    qw2[127, :] = np.float16(S_F16)

    EWp = np.zeros((VPAD, OUT), np.float32)
    EWp[:VOCAB] = emb @ W.T + b[None, :]

    flat = (np.arange(BATCH, dtype=np.int64)[:, None] * VPAD
            + k.astype(np.int64)).ravel()
    C = np.bincount(flat, minlength=BATCH * VPAD).reshape(BATCH, VPAD)
    assert C.max() <= 256, "count histogram overflows bf16 exact range"
    C = C.astype(np.float64)

    in_maps = []
    for core in range(N_CORES):
        v0 = core * VSH
        blocks = embT[:, v0:v0 + VSH].reshape(EMB, CSH, 128)
        e2 = np.zeros((128, NPAIR * 128), np.float16)
        e2[0:EMB] = blocks[:, 0::2, :].reshape(EMB, NPAIR * 128)
        e2[EMB:2 * EMB] = blocks[:, 1::2, :].reshape(EMB, NPAIR * 128)
        e2[127, :] = 1.0

        ew = EWp[v0:v0 + VSH].reshape(CSH, 128, OUT)
        stA = np.zeros((128, NQUAD, 9), np.float32)
        for qd in range(12):
            for j in range(4):
                stA[:, qd, 2 * j:2 * j + 2] = ew[4 * qd + j]
            stA[:, qd, 8] = 1.0
        stA[:, 12, 0:2] = ew[48]
        stA[:, 12, 2:4] = ew[49]
        stA[:, 12, 8] = 1.0

        # count layout matches A columns: [entry(128), chunk, batch]
        cc = (C[:, v0:v0 + VSH].reshape(BATCH, CSH, 128)
              .transpose(2, 1, 0).reshape(128, CSH * BATCH))
        ct = cc[:, 0:NPAIR_ACT * 256]
        cd = cc[:, NPAIR_ACT * 256:]
        with np.errstate(divide="ignore"):
            dd = np.where(cd > 0,
                          2.0 ** 7 * np.log2(np.maximum(cd, 1.0))
                          + K_SCHRAU * S_DELTA,
                          -np.inf)

        in_maps.append({
            "et": np.ascontiguousarray(e2),
            "qw": qw, "qw2": qw2,
            "stA": np.ascontiguousarray(
                stA.reshape(128, NQUAD * 9)).astype(ml_dtypes.bfloat16),
            "ct": np.ascontiguousarray(ct.astype(ml_dtypes.bfloat16)),
            "dd": np.ascontiguousarray(dd.astype(ml_dtypes.bfloat16)),
        })
    return in_maps


def _run_device(in_maps, **kwargs):
    from concourse.bass_utils import run_bass_kernel_spmd

    if "nc" not in _CACHE:
        _CACHE["nc"] = _build_nc()
    return run_bass_kernel_spmd(_CACHE["nc"], in_maps,
                                core_ids=list(range(N_CORES)), **kwargs)


def _unshard(res):
    P = np.zeros((9, 4 * BATCH), np.float64)
    for i in range(N_CORES):
        P += res.results[i]["o"].astype(np.float64)
    numer = np.zeros((OUT, BATCH), np.float64)
    denom = np.zeros(BATCH, np.float64)
    for j in range(4):
        numer += P[2 * j:2 * j + 2, j * BATCH:(j + 1) * BATCH]
        denom += P[8, j * BATCH:(j + 1) * BATCH]
    out = (numer / denom[None, :]).T
    return np.ascontiguousarray(out, dtype=np.float32)


def kernel(q, k, embeddings, W, b, **_unused):
    in_maps = _prep_inputs(q, k, embeddings, W, b)
    res = _run_device(in_maps)
    return _unshard(res)
